# revision 39
# baseline (speedup 1.0000x reference)
"""Trainium2 Bass kernel for one CLIP transformer layer (pre-LN causal
attention + GELU FFN), data-parallel over batch across 8 NeuronCores.

v2 redesign vs baseline (620-807us): keep the TensorE dense and warm.
  - all matmuls bf16 (fp8 fails the 2e-2 gate: measured 3e-2 in numpy sim)
  - attention: row-packed score pairs (two K=64 matmuls in disjoint row
    groups run concurrently), ONE paired exp per k-tile over a 2-bank PSUM
    tile, AV with appended-ones column; softmax normalization fully
    decoupled: raw po evacuated via ScalarE + DMA, row sums collected into
    a [16, s] tile, one reciprocal_approx_fast per q-chunk, selector-matmul
    broadcast, DVE multiply. No [1,512] single-lane reciprocals.
  - LDWEIGHTS amortization: every stationary streams 2 moving chunks
    (both q-chunks, or both halves of the V output).
  - LN: stats in bf16 (Sxx) / f32r (Sx) ones-matmuls, chain uses
    scalar_tensor_tensor + Sqrt + reciprocal_approx_fast; LN2 pipelined
    per q-chunk against the out-projection.
  - no keep_warm matmuls; ACT table prefetch via dummy activations.
"""
import math
from contextlib import ExitStack

import numpy as np

import concourse.bass as bass
import concourse.mybir as mybir
import concourse.tile as tile
from concourse import bacc
from concourse.bass_utils import run_bass_kernel_spmd

B, S, D, H, FF = 8, 1024, 1024, 16, 4096
DH = D // H
EPS = 1e-5
P = 128
QC = 512                 # q-chunk width == one fp32 PSUM bank
NEG = -1e10              # additive causal mask value

f32 = mybir.dt.float32
f32r = mybir.dt.float32r
bf16 = mybir.dt.bfloat16

ALU = mybir.AluOpType
ACTF = mybir.ActivationFunctionType

TRACE = False            # set by test.py for profiled runs
LAST_RESULTS = None      # BassKernelResults of the most recent run


class _Pool:
    """A tile pool with an explicit close() so SBUF is reclaimed mid-kernel
    (TileContext queue allocation mode reuses released ranges FIFO)."""

    def __init__(self, tc, **kw):
        self._cm = tc.tile_pool(**kw)
        self.pool = self._cm.__enter__()

    def tile(self, *a, **kw):
        if "name" not in kw:
            kw["name"] = kw.get("tag") or "t"
        return self.pool.tile(*a, **kw)

    def close(self):
        self._cm.__exit__(None, None, None)


def _layernorm(nc, tc, x_t, h_t, dc, cols, ones_sx, ones_p1b, ones_33,
               eps_sb, name):
    """LayerNorm over the partition (feature) axis of x_t [128, dc, s] for
    the column range `cols`, writing h_t = (x - mu) * rstd (bf16, same
    layout). gamma/beta are folded into the downstream weights on the host.

    Stats: Sx via f32r ones-matmul on x, Sxx via bf16 ones-matmul on x^2
    (DVE square). Chain: m = d*Sxx - Sx^2 (stt), sd = sqrt(m/d^2 + eps)
    (ACT), rstd = 1/sd (reciprocal_approx_fast), m2 = (Sx/d)*rstd (stt),
    apply h = x*rstd - m2 (2 DVE ops per chunk)."""
    d = dc * P
    c0, c1 = cols
    w = c1 - c0
    nq = w // QC
    with tc.tile_pool(name=name + "_sq", bufs=dc) as sqp, \
         tc.tile_pool(name=name + "_ch", bufs=1) as chp, \
         tc.tile_pool(name=name + "_ct", bufs=2) as ctp, \
         tc.tile_pool(name=name + "_ap", bufs=2) as app, \
         tc.tile_pool(name=name + "_ps", bufs=1, space="PSUM") as lps, \
         tc.tile_pool(name=name + "_pb", bufs=min(2, (c1 - c0) // QC),
                      space="PSUM") as bps:
        ps_st = lps.tile([33, w], f32, tag="st")   # row 0: Sx, row 32: Sxx
        st_sb = chp.tile([33, w], f32r, tag="stsb")
        xsqs = []
        for c in range(dc):
            xsq = sqp.tile([P, w], bf16, tag="xsq")
            nc.vector.tensor_mul(xsq, x_t[:, c, c0:c1], x_t[:, c, c0:c1])
            xsqs.append(xsq)
        rstd = chp.tile([P, w], f32, tag="rstd")
        m2 = chp.tile([P, w], f32, tag="m2")
        # per q-chunk: stats matmuls, then the scalar chain, then apply —
        # so chunk 0's chain overlaps chunk 1's stats and the consumer of
        # chunk 0's h_t can start early
        for q in range(nq):
            sl = slice(q * QC, (q + 1) * QC)
            for c in range(dc):
                nc.tensor.matmul(ps_st[0:1, sl], ones_sx,
                                 x_t[:, c, c0 + q * QC:c0 + (q + 1) * QC],
                                 start=(c == 0), stop=(c == dc - 1))
                nc.tensor.matmul(ps_st[32:33, sl], ones_p1b, xsqs[c][:, sl],
                                 start=(c == 0), stop=(c == dc - 1))
            # DVE copy with f32r output dtype: walrus requires f32r matmul
            # operands to come from an op that "rounds" to f32r (stt/DMA
            # do, ACT copy does not)
            nc.vector.tensor_scalar_add(st_sb[:, sl], ps_st[:, sl], 0.0)
            pbx = bps.tile([P, QC], f32, tag="pbx")
            pbxx = bps.tile([P, QC], f32, tag="pbxx")
            nc.tensor.matmul(pbx, ones_33[0:1, :], st_sb[0:1, sl],
                             start=True, stop=True)
            nc.tensor.matmul(pbxx, ones_33[32:33, :], st_sb[32:33, sl],
                             start=True, stop=True)
            a2 = ctp.tile([P, QC], f32, tag="a2")
            nc.scalar.activation(a2, pbx, ACTF.Square)
            m = ctp.tile([P, QC], f32, tag="m")
            nc.vector.scalar_tensor_tensor(m, pbxx, float(d), a2,
                                           op0=ALU.mult, op1=ALU.subtract)
            # rstd = (m/d^2 + eps)^-1/2  (argument is positive, so the
            # abs variant of reciprocal-sqrt computes exactly this)
            nc.scalar.activation(rstd[:, sl], m, ACTF.Abs_reciprocal_sqrt,
                                 bias=eps_sb,
                                 scale=float(1.0 / (d * d)))
            nc.vector.scalar_tensor_tensor(m2[:, sl], pbx, float(1.0 / d),
                                           rstd[:, sl],
                                           op0=ALU.mult, op1=ALU.mult)
            for c in range(dc):
                a0, a1 = c0 + q * QC, c0 + (q + 1) * QC
                tmp = app.tile([P, QC], f32, tag="app")
                nc.vector.tensor_mul(tmp, x_t[:, c, a0:a1], rstd[:, sl])
                nc.vector.tensor_sub(h_t[:, c, a0:a1], tmp, m2[:, sl])


def build_nc(s=S):
    """Build the per-core Bass program (SPMD; identical on all 8 cores)."""
    dc = D // P              # feature chunks
    nq = s // QC             # q chunks
    kts = s // P             # k tiles
    nhp = H // 2             # head pairs
    nft = FF // P            # FFN hidden tiles
    kpq = QC // P            # k-tiles per q-chunk
    hh = QC // DH            # heads per 512-wide V chunk

    nc = bacc.Bacc()
    xT = nc.declare_dram_parameter("xT", [D, s], f32r, isOutput=False)
    xTb = nc.declare_dram_parameter("xTb", [D, s], bf16, isOutput=False)
    wqkT = nc.declare_dram_parameter("wqkT", [D, 2 * D], bf16, isOutput=False)
    wvT = nc.declare_dram_parameter("wvT", [D, D], bf16, isOutput=False)
    woT = nc.declare_dram_parameter("woT", [D, D], bf16, isOutput=False)
    w1T = nc.declare_dram_parameter("w1T", [D, FF], bf16, isOutput=False)
    w2T = nc.declare_dram_parameter("w2T", [FF, D], bf16, isOutput=False)
    bqk = nc.declare_dram_parameter("bqk", [P, 2 * dc], f32, isOutput=False)
    bo = nc.declare_dram_parameter("bo", [P, dc], f32, isOutput=False)
    b1 = nc.declare_dram_parameter("b1", [P, nft], f32, isOutput=False)
    b2 = nc.declare_dram_parameter("b2", [P, dc], f32, isOutput=False)
    mk = nc.declare_dram_parameter("mk", [P, P], f32, isOutput=False)
    selm = nc.declare_dram_parameter("selm", [H, nhp * P], f32r,
                                     isOutput=False)
    onesd = nc.declare_dram_parameter("onesd", [P, P], f32r, isOutput=False)
    onesb = nc.declare_dram_parameter("onesb", [P, P], bf16, isOutput=False)
    outT = nc.declare_dram_parameter("outT", [D, s], f32, isOutput=True)

    def chunked(t):
        return t.rearrange("(c p) n -> p c n", p=P)

    with tile.TileContext(nc, pool_alloc_mode="queue") as tc:
        with tc.tile_pool(name="glob", bufs=1) as g:
            ones_p1f = g.tile([P, 1], f32r)
            nc.sync.dma_start(out=ones_p1f, in_=onesd[:, 0:1])
            ones_p1b = g.tile([P, 1], bf16)
            nc.sync.dma_start(out=ones_p1b, in_=onesb[:, 0:1])
            ones_33 = g.tile([33, P], f32r)
            nc.sync.dma_start(out=ones_33, in_=onesd[0:33, :])
            mask_sb = g.tile([P, P], f32)
            nc.sync.dma_start(out=mask_sb, in_=mk[:, :])
            sel_sb = g.tile([H, nhp * P], f32r)
            nc.sync.dma_start(out=sel_sb, in_=selm[:, :])
            bqk_sb = g.tile([P, 2 * dc], f32)
            nc.sync.dma_start(out=bqk_sb, in_=bqk[:, :])
            bo_sb = g.tile([P, dc], f32)
            nc.sync.dma_start(out=bo_sb, in_=bo[:, :])
            b1_sb = g.tile([P, nft], f32)
            nc.sync.dma_start(out=b1_sb, in_=b1[:, :])
            b2_sb = g.tile([P, dc], f32)
            nc.sync.dma_start(out=b2_sb, in_=b2[:, :])
            dum = g.tile([P, 8], f32)
            nc.vector.memset(dum, 1.0)
            dumo = g.tile([P, 8], f32)
            eps_sb = g.tile([P, 1], f32)
            nc.vector.memset(eps_sb, float(EPS))

            # long-lived big buffers; opened in reverse close order
            # (pool releases must be LIFO)
            xap = _Pool(tc, name="xattn", bufs=1)
            xattnT = xap.tile([P, dc, s], f32r, tag="xattnT")
            h2p = _Pool(tc, name="h2", bufs=1)
            h2T = h2p.tile([P, dc, s], bf16, tag="h2T")
            orp = _Pool(tc, name="oraw", bufs=1)
            oraw = orp.tile([P, nhp, s], bf16, tag="oraw")
            vp = _Pool(tc, name="v", bufs=1)
            v_sb = vp.tile([P, kts, H, DH + 1], bf16, tag="v_sb")
            qkp = _Pool(tc, name="qk", bufs=1)
            qt_all = qkp.tile([P, nhp, s], bf16, tag="qt")
            kt_all = qkp.tile([P, nhp, s], bf16, tag="kt")
            sums16 = qkp.tile([H, s], bf16, tag="sums16")

            # ------------- LN1 + V, pipelined per q-chunk ----------------
            # V(st 0..3) depends only on h1T's first 512 columns, so it
            # starts right after LN1(q0) while LN1(q1) runs on DVE/ScalarE.
            h1p = _Pool(tc, name="h1", bufs=1)
            h1T = h1p.tile([P, dc, s], bf16, tag="h1T")
            xin = _Pool(tc, name="xin", bufs=1)
            xt = xin.tile([P, dc, s], bf16, tag="xt")
            xT_c0 = chunked(xTb)
            for qh in range(nq):
                for c in range(dc):
                    hsl = slice(qh * QC, (qh + 1) * QC)
                    nc.sync.dma_start(out=xt[:, c, hsl],
                                      in_=xT_c0[:, c, hsl])
            with tc.tile_pool(name="wv", bufs=1) as wvp, \
                 tc.tile_pool(name="vps", bufs=3, space="PSUM") as vps:
                wv_sb = wvp.tile([P, dc, D], bf16)
                wv_ch = chunked(wvT)
                for c in range(dc):
                    nc.sync.dma_start(out=wv_sb[:, c, :], in_=wv_ch[:, c, :])

                def v_st(st):
                    pv = [vps.tile([P, QC], f32, tag="pv", name="pv")
                          for _ in range(2)]
                    for c in range(dc):
                        for vc in range(2):
                            nc.tensor.matmul(
                                pv[vc], h1T[:, c, st * P:(st + 1) * P],
                                wv_sb[:, c, vc * QC:(vc + 1) * QC],
                                start=(c == 0), stop=(c == dc - 1))
                    for vc in range(2):
                        nc.scalar.copy(
                            v_sb[:, st, vc * hh:(vc + 1) * hh, 0:DH],
                            pv[vc].rearrange("p (h e) -> p h e", h=hh))

                _layernorm(nc, tc, xt, h1T, dc, (0, QC), ones_p1b,
                           ones_p1b, ones_33, eps_sb, "ln1q0")
                for st in range(kts // 2):
                    v_st(st)
                _layernorm(nc, tc, xt, h1T, dc, (QC, 2 * QC), ones_p1b,
                           ones_p1b, ones_33, eps_sb, "ln1q1")
                for st in range(kts // 2, kts):
                    v_st(st)
                nc.sync.dma_start(
                    out=v_sb[:, :, :, DH:DH + 1],
                    in_=onesb[:, 0:kts * H].rearrange(
                        "p (k h o) -> p k h o", k=kts, h=H))
            xin.close()

            # -- Q/K projections interleaved with q0 attention head pairs --
            # (the dense QK matmuls fill the PE while ScalarE runs q0 exps)
            wqk_ch = chunked(wqkT)
            wo_ch = chunked(woT)
            xT_ch = chunked(xT)
            with tc.tile_pool(name="stg", bufs=4) as stp, \
                 tc.tile_pool(name="at", bufs=4) as atp, \
                 tc.tile_pool(name="nrm", bufs=2) as nrmp, \
                 tc.tile_pool(name="wo", bufs=2) as wop, \
                 tc.tile_pool(name="xres", bufs=3) as xrp:
                sps = _Pool(tc, name="sps", bufs=2, space="PSUM")
                ops = _Pool(tc, name="ops", bufs=2, space="PSUM")

                def attn_hp(q, hp):
                    sl = slice(q * QC, (q + 1) * QC)
                    nkt = (q + 1) * kpq
                    po = [ops.tile([DH + 1, QC], f32, tag="po",
                                   name="po") for _ in range(2)]
                    cache = {}

                    def scores(ki):
                        r = ki * P - q * QC
                        c0 = max(r, 0)
                        w = QC - c0
                        qsl = slice(q * QC + c0, (q + 1) * QC)
                        ps = sps.tile([P, 2, QC], f32, tag="ps", name="ps")
                        for hb in range(2):
                            hsl = slice(hb * DH, (hb + 1) * DH)
                            nc.tensor.matmul(
                                ps[:, hb, 0:w],
                                kt_all[hsl, hp, ki * P:(ki + 1) * P],
                                qt_all[hsl, hp, qsl],
                                start=True, stop=True)
                        if r >= 0:
                            for hb in range(2):
                                nc.vector.tensor_add(
                                    ps[:, hb, 0:P], ps[:, hb, 0:P], mask_sb)
                        at = atp.tile([P, 2, QC], bf16, tag="at", name="at")
                        nc.scalar.activation(at[:, :, 0:w], ps[:, :, 0:w],
                                             ACTF.Exp)
                        cache[ki] = (at, c0, w)

                    scores(0)
                    for ki in range(nkt):
                        if ki + 1 < nkt:
                            scores(ki + 1)
                        at, c0, w = cache.pop(ki)
                        for hb in range(2):
                            nc.tensor.matmul(
                                po[hb][:, c0:QC],
                                v_sb[:, ki, 2 * hp + hb, :],
                                at[:, hb, 0:w],
                                start=(ki == 0), stop=(ki == nkt - 1))
                    for hb in range(2):
                        stg = stp.tile([DH + 1, QC], bf16, tag="stg",
                                       name="stg")
                        nc.vector.tensor_scalar_add(stg, po[hb], 0.0)
                        nc.sync.dma_start(
                            out=oraw[hb * DH:(hb + 1) * DH, hp, sl],
                            in_=stg[0:DH, :])
                        hx = 2 * hp + hb
                        nc.sync.dma_start(out=sums16[hx:hx + 1, sl],
                                          in_=stg[DH:DH + 1, :])

                def norm_q(q, bpool):
                    sl = slice(q * QC, (q + 1) * QC)
                    sf = nrmp.tile([H, QC], f32, tag="sf", name="sf")
                    nc.vector.tensor_scalar_add(sf, sums16[:, sl], 0.0)
                    rinv = nrmp.tile([H, QC], f32, tag="rinv", name="rv")
                    nc.vector.reciprocal(rinv, sf)
                    rinv_r = nrmp.tile([H, QC], f32r, tag="rinvr",
                                       name="rvr")
                    nc.sync.dma_start(out=rinv_r, in_=rinv.bitcast(f32r))
                    for hp in range(nhp):
                        pb = bpool.tile([P, QC], f32, tag="pb", name="pb")
                        nc.tensor.matmul(pb, sel_sb[:, hp * P:(hp + 1) * P],
                                         rinv_r, start=True, stop=True)
                        nc.vector.tensor_mul(oraw[:, hp, sl],
                                             oraw[:, hp, sl], pb)

                def proj_chunk(q, ot, wpool, xpool, ppool):
                    sl = slice(q * QC, (q + 1) * QC)
                    wt = wpool.tile([P, dc, P], bf16, tag="wo", name="wt")
                    nc.sync.dma_start(
                        out=wt, in_=wo_ch[:, :, ot * P:(ot + 1) * P])
                    xr = xpool.tile([P, QC], f32r, tag="xr", name="xr")
                    nc.sync.dma_start(out=xr, in_=xT_ch[:, ot, sl])
                    pr = ppool.tile([P, QC], f32, tag="pr", name="pr")
                    for c in range(dc):
                        nc.tensor.matmul(pr, wt[:, c, :], oraw[:, c, sl],
                                         start=(c == 0), stop=(c == dc - 1))
                    nc.vector.scalar_tensor_tensor(
                        xattnT[:, ot, sl], pr, bo_sb[:, ot:ot + 1],
                        xr, op0=ALU.add, op1=ALU.add)

                wqkp = _Pool(tc, name="wqk", bufs=3)
                qps = _Pool(tc, name="qps", bufs=2, space="PSUM")

                def qk_hp(hp):
                    for which, dst in ((0, qt_all), (1, kt_all)):
                        wt = wqkp.tile([P, dc, P], bf16, tag="w", name="w")
                        o0 = which * D + hp * P
                        nc.sync.dma_start(out=wt,
                                          in_=wqk_ch[:, :, o0:o0 + P])
                        pq = [qps.tile([P, QC], f32, tag="pq", name="pq")
                              for _ in range(nq)]
                        for c in range(dc):
                            for q in range(nq):
                                sl = slice(q * QC, (q + 1) * QC)
                                nc.tensor.matmul(
                                    pq[q], wt[:, c, :], h1T[:, c, sl],
                                    start=(c == 0), stop=(c == dc - 1))
                        bcol = which * dc + hp
                        for q in range(nq):
                            sl = slice(q * QC, (q + 1) * QC)
                            nc.scalar.activation(
                                dst[:, hp, sl], pq[q], ACTF.Identity,
                                bias=bqk_sb[:, bcol:bcol + 1])

                for hp in range(nhp):
                    qk_hp(hp)
                    if hp == 0:
                        # prefetch the Exp ACT table set before the first
                        # q0 attention exp
                        nc.scalar.activation(dumo, dum, ACTF.Exp)
                    if hp >= 2:
                        attn_hp(0, hp - 2)
                attn_hp(0, nhp - 2)
                attn_hp(0, nhp - 1)
                qps.close()
                wqkp.close()

                bps = _Pool(tc, name="bps", bufs=1, space="PSUM")
                prs = _Pool(tc, name="prs", bufs=1, space="PSUM")
                # q0 normalization and projection ride one step behind the
                # q1 attention head pairs (their DVE/DMA latency hides
                # under the PE's attention work)
                for hp in range(nhp):
                    attn_hp(1, hp)
                    if hp == 0:
                        norm_q(0, bps)
                    else:
                        proj_chunk(0, hp - 1, wop, xrp, prs)
                proj_chunk(0, nhp - 1, wop, xrp, prs)
                norm_q(1, bps)
                # prefetch the LN2 (rsqrt) table set
                nc.scalar.activation(dumo, dum, ACTF.Abs_reciprocal_sqrt)
                prs.close()
                bps.close()
                ops.close()
                sps.close()
            h1p.close()
            qkp.close()
            vp.close()

            # --- out-projection q1 with LN2(q0) tucked behind chunk 0 ----
            with tc.tile_pool(name="wo2", bufs=2) as wop2, \
                 tc.tile_pool(name="xres2", bufs=3) as xrp2, \
                 tc.tile_pool(name="prs2", bufs=3, space="PSUM") as prs2:
                proj_chunk(1, 0, wop2, xrp2, prs2)
                _layernorm(nc, tc, xattnT, h2T, dc, (0, QC), ones_p1f,
                           ones_p1b, ones_33, eps_sb, "ln2q0")
                for ot in range(1, dc):
                    proj_chunk(1, ot, wop2, xrp2, prs2)
            orp.close()
            _layernorm(nc, tc, xattnT, h2T, dc, (QC, 2 * QC), ones_p1f,
                       ones_p1b, ones_33, eps_sb, "ln2q1")

            # ---------------- FFN ----------------
            with tc.tile_pool(name="aff", bufs=2 * nft) as affp, \
                 tc.tile_pool(name="w1", bufs=3) as w1p, \
                 tc.tile_pool(name="aps", bufs=4, space="PSUM") as aps:
                w1_ch = chunked(w1T)
                a_tiles = [[None] * nft for _ in range(nq)]
                for fc in range(nft):
                    wt = w1p.tile([P, dc, P], bf16, tag="w1")
                    nc.sync.dma_start(
                        out=wt, in_=w1_ch[:, :, fc * P:(fc + 1) * P])
                    pa = [aps.tile([P, QC], f32, tag="pa", name="pa")
                          for _ in range(nq)]
                    for c in range(dc):
                        for q in range(nq):
                            sl = slice(q * QC, (q + 1) * QC)
                            nc.tensor.matmul(pa[q], wt[:, c, :],
                                             h2T[:, c, sl],
                                             start=(c == 0),
                                             stop=(c == dc - 1))
                    for q in range(nq):
                        a = affp.tile([P, QC], bf16, tag="a")
                        nc.scalar.activation(a, pa[q], ACTF.Gelu_apprx_tanh,
                                             bias=b1_sb[:, fc:fc + 1])
                        a_tiles[q][fc] = a

                with tc.tile_pool(name="w2", bufs=2) as w2p, \
                     tc.tile_pool(name="yout", bufs=4) as youtp, \
                     tc.tile_pool(name="yps", bufs=4, space="PSUM") as yps:
                    w2_ch = chunked(w2T)
                    for do in range(dc):
                        wt2 = w2p.tile([P, nft, P], bf16, tag="w2")
                        nc.sync.dma_start(
                            out=wt2, in_=w2_ch[:, :, do * P:(do + 1) * P])
                        py = [yps.tile([P, QC], f32, tag="py", name="py")
                              for _ in range(nq)]
                        for fi in range(nft):
                            for q in range(nq):
                                nc.tensor.matmul(py[q], wt2[:, fi, :],
                                                 a_tiles[q][fi],
                                                 start=(fi == 0),
                                                 stop=(fi == nft - 1))
                        for q in range(nq):
                            sl = slice(q * QC, (q + 1) * QC)
                            y = youtp.tile([P, QC], f32, tag="y")
                            nc.vector.scalar_tensor_tensor(
                                y, py[q], b2_sb[:, do:do + 1],
                                xattnT[:, do, sl], op0=ALU.add, op1=ALU.add)
                            nc.sync.dma_start(
                                out=outT[do * P:(do + 1) * P, sl], in_=y)
            h2p.close()
            xap.close()

    nc.compile()
    return nc


def prep_inputs(x, ln1_g, ln1_b, w_qkv, b_qkv, w_o, b_o, ln2_g, ln2_b,
                w1, b1, w2, b2, s=S):
    """Host-side preprocessing: LN gamma/beta folding, Q-scale folding,
    V-bias folding, transposes, per-tile bias layouts."""
    f = np.float32
    x = np.asarray(x, f)
    ln1_g, ln1_b = np.asarray(ln1_g, f), np.asarray(ln1_b, f)
    ln2_g, ln2_b = np.asarray(ln2_g, f), np.asarray(ln2_b, f)
    w_qkv, b_qkv = np.asarray(w_qkv, f), np.asarray(b_qkv, f)
    w_o, b_o = np.asarray(w_o, f), np.asarray(b_o, f)
    w1, b1 = np.asarray(w1, f), np.asarray(b1, f)
    w2, b2 = np.asarray(w2, f), np.asarray(b2, f)

    wqkv_e = w_qkv * ln1_g[None, :]
    bqkv_e = b_qkv + w_qkv @ ln1_b
    sc = f(1.0 / math.sqrt(DH))
    wq = wqkv_e[0:D] * sc
    bq = bqkv_e[0:D] * sc
    wk, bk = wqkv_e[D:2 * D], bqkv_e[D:2 * D]
    wv, bv = wqkv_e[2 * D:], bqkv_e[2 * D:]

    dcn = D // P
    nhp = H // 2
    import ml_dtypes
    npb = ml_dtypes.bfloat16

    # softmax-normalization broadcast selectors: for head pair hp, column
    # block m in [0,128): row k == head index 2*hp + (m >= 64)
    sel = np.zeros((H, nhp * P), f)
    for hp in range(nhp):
        sel[2 * hp, hp * P:hp * P + DH] = 1.0
        sel[2 * hp + 1, hp * P + DH:(hp + 1) * P] = 1.0

    common = {
        "wqkT": np.ascontiguousarray(np.concatenate([wq, wk], 0).T).astype(npb),
        "wvT": np.ascontiguousarray(wv.T).astype(npb),
        "woT": np.ascontiguousarray(w_o.T).astype(npb),
        "w1T": np.ascontiguousarray((w1 * ln2_g[None, :]).T).astype(npb),
        "w2T": np.ascontiguousarray(w2.T).astype(npb),
        "bqk": np.ascontiguousarray(
            np.concatenate([bq, bk]).reshape(2 * dcn, P).T),
        "bo": np.ascontiguousarray((b_o + w_o @ bv).reshape(dcn, P).T),
        "b1": np.ascontiguousarray(
            (b1 + w1 @ ln2_b).reshape(FF // P, P).T),
        "b2": np.ascontiguousarray(b2.reshape(dcn, P).T),
        "mk": np.where(np.arange(P)[:, None] > np.arange(P)[None, :],
                       f(NEG), f(0.0)),
        "selm": sel,
        "onesd": np.ones((P, P), f),
        "onesb": np.ones((P, P), npb),
    }
    in_maps = []
    for b in range(x.shape[0]):
        m = dict(common)
        xTb = np.ascontiguousarray(x[b, :s].T)
        m["xT"] = xTb
        m["xTb"] = xTb.astype(npb)
        in_maps.append(m)
    return in_maps


_NC_CACHE = {}


def kernel(**inputs) -> np.ndarray:
    global LAST_RESULTS
    if S not in _NC_CACHE:
        _NC_CACHE[S] = build_nc(S)
    nc = _NC_CACHE[S]
    in_maps = prep_inputs(**inputs)
    res = run_bass_kernel_spmd(nc, in_maps, core_ids=list(range(B)),
                               trace=TRACE)
    LAST_RESULTS = res
    out = np.stack([res.results[b]["outT"].T for b in range(B)])
    return np.ascontiguousarray(out.astype(np.float32))
